# revision 6
# baseline (speedup 1.0000x reference)
"""Trainium2 Bass kernel for out = x * w (column-wise scale).

x: [16384, 4096] f32, w: [4096] f32 -> out[i, j] = x[i, j] * w[j].

Data-parallel across 8 NeuronCores: each core handles a [2048, 4096] row
shard of x; w is replicated. Per core the shard streams through SBUF as
16 tiles of [128, 4096] (2 MiB DMAs, 16 KiB contiguous per partition),
multiplied in place on the vector engine against a [128, 4096] broadcast
copy of w (stride-0 DMA source), and stored back.

Raw Bass (no Tile framework): the walrus build in this toolchain allows
only ONE semaphore wait per compute instruction, so all waits are emitted
as standalone wait_ge instructions on each engine queue. Loads run SLOTS-1
tiles ahead of stores, so slot-release waits never actually block.
"""

import sys

for _p in ("/opt/trn_rl_repo",):
    if _p not in sys.path:
        sys.path.insert(0, _p)

from contextlib import ExitStack

import numpy as np

import concourse.bass as bass
import concourse.mybir as mybir
from concourse.bass_utils import run_bass_kernel_spmd

ROWS = 16384
SIZE = 4096
N_CORES = 8
ROWS_PER_CORE = ROWS // N_CORES  # 2048
P = 128                          # SBUF partitions
N_TILES = ROWS_PER_CORE // P     # 16 tiles of [128, 4096]
SLOTS = 9                        # SBUF ring depth (9*16KiB + w 16KiB per partition)

_nc_cache = None


def _build() -> bass.Bass:
    f32 = mybir.dt.float32
    nc = bass.Bass()
    x = nc.declare_dram_parameter("x", [ROWS_PER_CORE, SIZE], f32, isOutput=False)
    w = nc.declare_dram_parameter("w", [SIZE], f32, isOutput=False)
    y = nc.declare_dram_parameter("y", [ROWS_PER_CORE, SIZE], f32, isOutput=True)

    with ExitStack() as ctx:
        w_tile = ctx.enter_context(nc.sbuf_tensor([P, SIZE], f32))
        tbuf = ctx.enter_context(nc.sbuf_tensor([P, SLOTS * SIZE], f32))
        w_sem = ctx.enter_context(nc.semaphore("w_sem"))
        dve_sem = ctx.enter_context(nc.semaphore("dve_sem"))
        in_sems = [
            ctx.enter_context(nc.semaphore(f"in_sem{a}")) for a in range(SLOTS)
        ]
        out_sems = [
            ctx.enter_context(nc.semaphore(f"out_sem{a}")) for a in range(SLOTS)
        ]
        block = ctx.enter_context(nc.Block())

        def slot(a):
            return tbuf[:, a * SIZE : (a + 1) * SIZE]

        AHEAD = SLOTS - 2  # loads lead stores; slot-release waits stay slack

        @block.gpsimd
        def _(g: bass.BassEngine):
            g.dma_start(
                out=w_tile[:], in_=w[None, :].partition_broadcast(P)
            ).then_inc(w_sem, 16)
            for i in range(min(AHEAD, N_TILES)):
                g.dma_start(
                    out=slot(i % SLOTS), in_=x[i * P : (i + 1) * P, :]
                ).then_inc(in_sems[i % SLOTS], 16)
            for i in range(N_TILES):
                j = i + AHEAD
                if j < N_TILES:
                    b = j % SLOTS
                    if j >= SLOTS:
                        # slot b last read by store(j - SLOTS); all j//SLOTS
                        # earlier stores of this slot must have completed
                        g.wait_ge(out_sems[b], 16 * (j // SLOTS))
                    g.dma_start(
                        out=slot(b), in_=x[j * P : (j + 1) * P, :]
                    ).then_inc(in_sems[b], 16)
                a = i % SLOTS
                g.wait_ge(dve_sem, i + 1)
                g.dma_start(
                    out=y[i * P : (i + 1) * P, :], in_=slot(a)
                ).then_inc(out_sems[a], 16)

        @block.vector
        def _(v: bass.BassEngine):
            v.wait_ge(w_sem, 16)
            for i in range(N_TILES):
                a = i % SLOTS
                v.wait_ge(in_sems[a], 16 * (i // SLOTS + 1))
                v.tensor_mul(slot(a), slot(a), w_tile[:]).then_inc(dve_sem, 1)

    return nc


def _run(x: np.ndarray, w: np.ndarray, **spmd_kwargs):
    global _nc_cache
    if _nc_cache is None:
        _nc_cache = _build()
    x = np.ascontiguousarray(x, dtype=np.float32)
    w = np.ascontiguousarray(w, dtype=np.float32)
    in_maps = [
        {"x": x[i * ROWS_PER_CORE : (i + 1) * ROWS_PER_CORE], "w": w}
        for i in range(N_CORES)
    ]
    return run_bass_kernel_spmd(_nc_cache, in_maps, list(range(N_CORES)), **spmd_kwargs)


def kernel(x: np.ndarray, w: np.ndarray) -> np.ndarray:
    res = _run(x, w)
    return np.concatenate([res.results[i]["y"] for i in range(N_CORES)], axis=0)


# revision 7
# speedup vs baseline: 1.1624x; 1.1624x over previous
"""Trainium2 Bass kernel for out = x * w (column-wise scale).

x: [16384, 4096] f32, w: [4096] f32 -> out[i, j] = x[i, j] * w[j].

Data-parallel across 8 NeuronCores: each core handles a [2048, 4096] row
shard of x; w is replicated. Per core the shard streams through SBUF as
16 tiles of [128, 4096] (2 MiB DMAs, 16 KiB contiguous per partition),
multiplied in place on the vector engine against a [128, 4096] broadcast
copy of w (stride-0 DMA source), and stored back.

Raw Bass (no Tile framework): the walrus build in this toolchain allows
only ONE semaphore wait per compute instruction, so all waits are emitted
as standalone wait_ge instructions on each engine queue. Loads run SLOTS-1
tiles ahead of stores, so slot-release waits never actually block.
"""

import sys

for _p in ("/opt/trn_rl_repo",):
    if _p not in sys.path:
        sys.path.insert(0, _p)

from contextlib import ExitStack

import numpy as np

import concourse.bass as bass
import concourse.mybir as mybir
from concourse.bass_utils import run_bass_kernel_spmd

ROWS = 16384
SIZE = 4096
N_CORES = 8
ROWS_PER_CORE = ROWS // N_CORES  # 2048
P = 128                          # SBUF partitions
N_TILES = ROWS_PER_CORE // P     # 16 tiles of [128, 4096]
SLOTS = 9                        # SBUF ring depth (9*16KiB + w 16KiB per partition)

_nc_cache = None


def _build() -> bass.Bass:
    f32 = mybir.dt.float32
    nc = bass.Bass()
    x = nc.declare_dram_parameter("x", [ROWS_PER_CORE, SIZE], f32, isOutput=False)
    w = nc.declare_dram_parameter("w", [SIZE], f32, isOutput=False)
    y = nc.declare_dram_parameter("y", [ROWS_PER_CORE, SIZE], f32, isOutput=True)

    with ExitStack() as ctx:
        w_tile = ctx.enter_context(nc.sbuf_tensor([P, SIZE], f32))
        tbuf = ctx.enter_context(nc.sbuf_tensor([P, SLOTS * SIZE], f32))
        w_sem = ctx.enter_context(nc.semaphore("w_sem"))
        dve_sem = ctx.enter_context(nc.semaphore("dve_sem"))
        in_sems = [
            ctx.enter_context(nc.semaphore(f"in_sem{a}")) for a in range(SLOTS)
        ]
        out_sems = [
            ctx.enter_context(nc.semaphore(f"out_sem{a}")) for a in range(SLOTS)
        ]
        block = ctx.enter_context(nc.Block())

        def slot(a):
            return tbuf[:, a * SIZE : (a + 1) * SIZE]

        # Loads on the SWDGE ring (Pool queue), stores on the independent
        # HWDGE ring (sync/SP queue), w broadcast also via HWDGE so the
        # load ring starts streaming x immediately.
        @block.gpsimd
        def _(g: bass.BassEngine):
            for j in range(N_TILES):
                b = j % SLOTS
                if j >= SLOTS:
                    # slot b last read by store(j - SLOTS); all j//SLOTS
                    # earlier stores of this slot must have completed
                    g.wait_ge(out_sems[b], 16 * (j // SLOTS))
                g.dma_start(
                    out=slot(b), in_=x[j * P : (j + 1) * P, :]
                ).then_inc(in_sems[b], 16)

        @block.sync
        def _(s: bass.BassEngine):
            s.dma_start(
                out=w_tile[:], in_=w[None, :].partition_broadcast(P)
            ).then_inc(w_sem, 16)
            for i in range(N_TILES):
                a = i % SLOTS
                s.wait_ge(dve_sem, i + 1)
                s.dma_start(
                    out=y[i * P : (i + 1) * P, :], in_=slot(a)
                ).then_inc(out_sems[a], 16)

        @block.vector
        def _(v: bass.BassEngine):
            v.wait_ge(w_sem, 16)
            for i in range(N_TILES):
                a = i % SLOTS
                v.wait_ge(in_sems[a], 16 * (i // SLOTS + 1))
                v.tensor_mul(slot(a), slot(a), w_tile[:]).then_inc(dve_sem, 1)

    return nc


def _run(x: np.ndarray, w: np.ndarray, **spmd_kwargs):
    global _nc_cache
    if _nc_cache is None:
        _nc_cache = _build()
    x = np.ascontiguousarray(x, dtype=np.float32)
    w = np.ascontiguousarray(w, dtype=np.float32)
    in_maps = [
        {"x": x[i * ROWS_PER_CORE : (i + 1) * ROWS_PER_CORE], "w": w}
        for i in range(N_CORES)
    ]
    return run_bass_kernel_spmd(_nc_cache, in_maps, list(range(N_CORES)), **spmd_kwargs)


def kernel(x: np.ndarray, w: np.ndarray) -> np.ndarray:
    res = _run(x, w)
    return np.concatenate([res.results[i]["y"] for i in range(N_CORES)], axis=0)


# revision 8
# speedup vs baseline: 1.1993x; 1.0317x over previous
"""Trainium2 Bass kernel for out = x * w (column-wise scale).

x: [16384, 4096] f32, w: [4096] f32 -> out[i, j] = x[i, j] * w[j].

Data-parallel across 8 NeuronCores: each core handles a [2048, 4096] row
shard of x; w is replicated. Per core the shard streams through SBUF as
16 tiles of [128, 4096] (2 MiB DMAs, 16 KiB contiguous per partition),
multiplied in place on the vector engine against a [128, 4096] broadcast
copy of w (stride-0 DMA source), and stored back.

Raw Bass (no Tile framework): the walrus build in this toolchain allows
only ONE semaphore wait per compute instruction, so all waits are emitted
as standalone wait_ge instructions on each engine queue. Loads run SLOTS-1
tiles ahead of stores, so slot-release waits never actually block.
"""

import sys

for _p in ("/opt/trn_rl_repo",):
    if _p not in sys.path:
        sys.path.insert(0, _p)

from contextlib import ExitStack

import numpy as np

import concourse.bass as bass
import concourse.mybir as mybir
from concourse.bass_utils import run_bass_kernel_spmd

ROWS = 16384
SIZE = 4096
N_CORES = 8
ROWS_PER_CORE = ROWS // N_CORES  # 2048
P = 128                          # SBUF partitions
N_TILES = ROWS_PER_CORE // P     # 16 tiles of [128, 4096]
SLOTS = 9                        # SBUF ring depth (9*16KiB + w 16KiB per partition)

_nc_cache = None


def _build() -> bass.Bass:
    f32 = mybir.dt.float32
    nc = bass.Bass()
    x = nc.declare_dram_parameter("x", [ROWS_PER_CORE, SIZE], f32, isOutput=False)
    w = nc.declare_dram_parameter("w", [SIZE], f32, isOutput=False)
    y = nc.declare_dram_parameter("y", [ROWS_PER_CORE, SIZE], f32, isOutput=True)

    with ExitStack() as ctx:
        w_tile = ctx.enter_context(nc.sbuf_tensor([P, SIZE], f32))
        tbuf = ctx.enter_context(nc.sbuf_tensor([P, SLOTS * SIZE], f32))
        w_sem = ctx.enter_context(nc.semaphore("w_sem"))
        dve_sem = ctx.enter_context(nc.semaphore("dve_sem"))
        in_sems = [
            ctx.enter_context(nc.semaphore(f"in_sem{a}")) for a in range(SLOTS)
        ]
        out_sems = [
            ctx.enter_context(nc.semaphore(f"out_sem{a}")) for a in range(SLOTS)
        ]
        block = ctx.enter_context(nc.Block())

        def slot(a):
            return tbuf[:, a * SIZE : (a + 1) * SIZE]

        # Two independent DMA rings, balanced end-to-end: even tiles load
        # on the SWDGE ring (Pool queue) and store on the HWDGE ring
        # (sync/SP queue); odd tiles the reverse. Each ring carries half
        # the loads and half the stores so both drain together instead of
        # leaving a store-only half-bandwidth tail. Interleave stores
        # between loads on each queue; loads run ahead so the dve waits
        # in front of stores are the only ones that ever block.
        def emit_queue(q: bass.BassEngine, load_par: int):
            if load_par == 1:
                # HWDGE ring also carries the w broadcast, first
                q.dma_start(
                    out=w_tile[:], in_=w[None, :].partition_broadcast(P)
                ).then_inc(w_sem, 16)
            loads = list(range(load_par, N_TILES, 2))
            stores = list(range(1 - load_par, N_TILES, 2))
            li = si = 0
            while li < len(loads) or si < len(stores):
                # issue loads eagerly, up to AHEAD tiles past the last
                # store this queue has issued
                while li < len(loads) and (
                    si >= len(stores) or loads[li] < stores[si] + AHEAD
                ):
                    j = loads[li]
                    b = j % SLOTS
                    if j >= SLOTS:
                        # slot b last read by store(j - SLOTS)
                        q.wait_ge(out_sems[b], 16 * (j // SLOTS))
                    q.dma_start(
                        out=slot(b), in_=x[j * P : (j + 1) * P, :]
                    ).then_inc(in_sems[b], 16)
                    li += 1
                if si < len(stores):
                    i = stores[si]
                    a = i % SLOTS
                    q.wait_ge(dve_sem, i + 1)
                    q.dma_start(
                        out=y[i * P : (i + 1) * P, :], in_=slot(a)
                    ).then_inc(out_sems[a], 16)
                    si += 1

        AHEAD = SLOTS - 2

        @block.gpsimd
        def _(g: bass.BassEngine):
            emit_queue(g, 0)

        @block.sync
        def _(s: bass.BassEngine):
            emit_queue(s, 1)

        @block.vector
        def _(v: bass.BassEngine):
            v.wait_ge(w_sem, 16)
            for i in range(N_TILES):
                a = i % SLOTS
                v.wait_ge(in_sems[a], 16 * (i // SLOTS + 1))
                v.tensor_mul(slot(a), slot(a), w_tile[:]).then_inc(dve_sem, 1)

    return nc


def _run(x: np.ndarray, w: np.ndarray, **spmd_kwargs):
    global _nc_cache
    if _nc_cache is None:
        _nc_cache = _build()
    x = np.ascontiguousarray(x, dtype=np.float32)
    w = np.ascontiguousarray(w, dtype=np.float32)
    in_maps = [
        {"x": x[i * ROWS_PER_CORE : (i + 1) * ROWS_PER_CORE], "w": w}
        for i in range(N_CORES)
    ]
    return run_bass_kernel_spmd(_nc_cache, in_maps, list(range(N_CORES)), **spmd_kwargs)


def kernel(x: np.ndarray, w: np.ndarray) -> np.ndarray:
    res = _run(x, w)
    return np.concatenate([res.results[i]["y"] for i in range(N_CORES)], axis=0)


# revision 11
# speedup vs baseline: 1.3601x; 1.1341x over previous
"""Trainium2 Bass kernel for out = x * w (column-wise scale).

x: [16384, 4096] f32, w: [4096] f32 -> out[i, j] = x[i, j] * w[j].

Data-parallel across 8 NeuronCores: each core handles a [2048, 4096] row
shard of x; w is replicated. Per core the shard streams through SBUF as
16 tiles of [128, 4096] (2 MiB DMAs, 16 KiB contiguous per partition),
multiplied in place on the vector engine against a [128, 4096] broadcast
copy of w (stride-0 DMA source), and stored back.

Raw Bass (no Tile framework): the walrus build in this toolchain allows
only ONE semaphore wait per compute instruction, so all waits are emitted
as standalone wait_ge instructions on each engine queue. Loads run SLOTS-1
tiles ahead of stores, so slot-release waits never actually block.
"""

import sys

for _p in ("/opt/trn_rl_repo",):
    if _p not in sys.path:
        sys.path.insert(0, _p)

from contextlib import ExitStack

import numpy as np

import concourse.bass as bass
import concourse.mybir as mybir
from concourse.bass_utils import run_bass_kernel_spmd

ROWS = 16384
SIZE = 4096
N_CORES = 8
ROWS_PER_CORE = ROWS // N_CORES  # 2048
P = 128                          # SBUF partitions
N_TILES = ROWS_PER_CORE // P     # 16 tiles of [128, 4096]
SLOTS = 9                        # SBUF ring depth (9*16KiB + w 16KiB per partition)

_nc_cache = None


def _build() -> bass.Bass:
    f32 = mybir.dt.float32
    nc = bass.Bass()
    x = nc.declare_dram_parameter("x", [ROWS_PER_CORE, SIZE], f32, isOutput=False)
    w = nc.declare_dram_parameter("w", [SIZE], f32, isOutput=False)
    y = nc.declare_dram_parameter("y", [ROWS_PER_CORE, SIZE], f32, isOutput=True)

    with ExitStack() as ctx:
        w_tile = ctx.enter_context(nc.sbuf_tensor([P, SIZE], f32))
        w_row = ctx.enter_context(nc.sbuf_tensor([1, SIZE], f32))
        ones_t = ctx.enter_context(nc.sbuf_tensor([1, P], f32))
        psum_w = ctx.enter_context(nc.psum_tensor([P, SIZE], f32))
        tbuf = ctx.enter_context(nc.sbuf_tensor([P, SLOTS * SIZE], f32))
        w_sem = ctx.enter_context(nc.semaphore("w_sem"))
        ones_sem = ctx.enter_context(nc.semaphore("ones_sem"))
        pe_sem = ctx.enter_context(nc.semaphore("pe_sem"))
        wcopy_sem = ctx.enter_context(nc.semaphore("wcopy_sem"))
        dve_sem = ctx.enter_context(nc.semaphore("dve_sem"))
        in_sems = [
            ctx.enter_context(nc.semaphore(f"in_sem{a}")) for a in range(SLOTS)
        ]
        out_sems = [
            ctx.enter_context(nc.semaphore(f"out_sem{a}")) for a in range(SLOTS)
        ]
        block = ctx.enter_context(nc.Block())

        def slot(a):
            return tbuf[:, a * SIZE : (a + 1) * SIZE]

        # Two independent DMA rings, balanced end-to-end: even tiles load
        # on the SWDGE ring (Pool queue) and store on the HWDGE ring
        # (sync/SP queue); odd tiles the reverse. Each ring carries half
        # the loads and half the stores so both drain together instead of
        # leaving a store-only half-bandwidth tail. Interleave stores
        # between loads on each queue; loads run ahead so the dve waits
        # in front of stores are the only ones that ever block.
        def emit_queue(q: bass.BassEngine, load_par: int):
            if load_par == 1:
                # HWDGE ring carries the 16 KiB w row first (broadcast to
                # 128 partitions happens on-chip via a rank-1 PE matmul)
                q.dma_start(out=w_row[:], in_=w[None, :]).then_inc(w_sem, 16)
            loads = list(range(load_par, N_TILES, 2))
            stores = list(range(1 - load_par, N_TILES, 2))
            li = si = 0
            while li < len(loads) or si < len(stores):
                # issue loads eagerly, up to AHEAD tiles past the last
                # store this queue has issued
                while li < len(loads) and (
                    si >= len(stores) or loads[li] < stores[si] + AHEAD
                ):
                    j = loads[li]
                    b = j % SLOTS
                    if j >= SLOTS:
                        # slot b last read by store(j - SLOTS)
                        q.wait_ge(out_sems[b], 16 * (j // SLOTS))
                    q.dma_start(
                        out=slot(b), in_=x[j * P : (j + 1) * P, :]
                    ).then_inc(in_sems[b], 16)
                    li += 1
                if si < len(stores):
                    i = stores[si]
                    a = i % SLOTS
                    q.wait_ge(dve_sem, i + 1)
                    q.dma_start(
                        out=y[i * P : (i + 1) * P, :], in_=slot(a)
                    ).then_inc(out_sems[a], 16)
                    si += 1

        AHEAD = SLOTS - 2

        @block.gpsimd
        def _(g: bass.BassEngine):
            emit_queue(g, 0)

        @block.sync
        def _(s: bass.BassEngine):
            emit_queue(s, 1)

        MM_N = 512  # one PSUM bank of f32 per matmul

        @block.tensor
        def _(t: bass.BassEngine):
            t.wait_ge(w_sem, 16)
            t.wait_ge(ones_sem, 1)
            for b in range(SIZE // MM_N):
                # psum_w[p, n] = ones[0, p] * w_row[0, n] — partition bcast
                t.matmul(
                    psum_w[:, b * MM_N : (b + 1) * MM_N],
                    ones_t[:],
                    w_row[:, b * MM_N : (b + 1) * MM_N],
                    start=True,
                    stop=True,
                ).then_inc(pe_sem, 1)

        @block.vector
        def _(v: bass.BassEngine):
            v.memset(ones_t[:], 1.0).then_inc(ones_sem, 1)
            v.wait_ge(pe_sem, SIZE // MM_N)
            v.tensor_copy(w_tile[:], psum_w[:]).then_inc(wcopy_sem, 1)
            v.wait_ge(wcopy_sem, 1)
            for i in range(N_TILES):
                a = i % SLOTS
                v.wait_ge(in_sems[a], 16 * (i // SLOTS + 1))
                v.tensor_mul(slot(a), slot(a), w_tile[:]).then_inc(dve_sem, 1)

    return nc


def _run(x: np.ndarray, w: np.ndarray, **spmd_kwargs):
    global _nc_cache
    if _nc_cache is None:
        _nc_cache = _build()
    x = np.ascontiguousarray(x, dtype=np.float32)
    w = np.ascontiguousarray(w, dtype=np.float32)
    in_maps = [
        {"x": x[i * ROWS_PER_CORE : (i + 1) * ROWS_PER_CORE], "w": w}
        for i in range(N_CORES)
    ]
    return run_bass_kernel_spmd(_nc_cache, in_maps, list(range(N_CORES)), **spmd_kwargs)


def kernel(x: np.ndarray, w: np.ndarray) -> np.ndarray:
    res = _run(x, w)
    return np.concatenate([res.results[i]["y"] for i in range(N_CORES)], axis=0)
